# revision 8
# baseline (speedup 1.0000x reference)
"""Multi-head attention (B=4, N=2048, D=1024, H=16) on 8 TRN2 NeuronCores.

Sharding: 8 cores = batch(4) x sequence-half(2). Each core computes the full
attention output for its 1024-token slice of one batch (all 16 heads), so the
final unshard is a pure gather. The only cross-core traffic is an AllGather of
K^T and V between the two cores of each batch pair.

Per-core pipeline (bf16 matmul operands, fp32 PSUM accumulation):
  1. Cast x / w_qkv / w_proj to bf16, stage to DRAM, and DMA-transpose back so
     contraction dims sit on SBUF partitions.
  2. QKV projection. Q^T and K^T are produced in [d_out, token] orientation
     (lhsT = w_qkv^T tile, rhs = x^T); V in natural [token, d] orientation
     (lhsT = x^T tile, rhs = w_qkv^T).
  3. AllGather K^T then V across the pair (k-token axis spans both halves).
  4. Attention per head-pair p: S^T = (QK^T)^T via row-paired matmuls
     (contraction = head_dim 64, two heads in array row halves), exp on
     ScalarE straight out of PSUM (logits are bounded, no max subtraction),
     then O^T and the softmax denominator via col-paired matmuls over the
     k axis. The all-ones denominator lhsT replicates each head's denominator
     across its 64 output partitions, so normalization is a single full-width
     reciprocal + multiply on VectorE.
  5. Output projection from the accumulated attout^T tiles, bias add, DMA out.
"""

import sys

for _p in ("/opt/trn_rl_repo",):
    if _p not in sys.path:
        sys.path.insert(0, _p)

import numpy as np

import concourse.bass as bass
import concourse.mybir as mybir
import concourse.tile as tile
from concourse import bacc
from concourse.bass_utils import run_bass_kernel_spmd

B, N, D, H, HD = 4, 2048, 1024, 16, 64
SCALE = HD ** -0.5
NL = N // 2  # tokens per core
NCORES = 8
RG = [[0, 1], [2, 3], [4, 5], [6, 7]]
F32 = mybir.dt.float32
BF16 = mybir.dt.bfloat16
EXP = mybir.ActivationFunctionType.Exp


def _emit(tc, aps):
    nc = tc.nc
    x_l, wqkv, wproj, bias, out = (
        aps["x_local"], aps["w_qkv"], aps["w_proj"], aps["b_proj"], aps["out"])
    x_blk, wqkv_blk, wproj_blk = aps["x_blk"], aps["wqkv_blk"], aps["wproj_blk"]
    cc_k, cc_v, k_g, v_g = aps["cc_k"], aps["cc_v"], aps["k_g"], aps["v_g"]

    persist1 = tc.alloc_tile_pool(name="persist1", bufs=1)
    psum = tc.alloc_tile_pool(name="psum", bufs=1, space="PSUM")

    # ---- Phase A: load fp32, cast bf16, stage to DRAM in column-blocked
    # layout (one [rows, 128] contiguous block per k-tile) so the later
    # DMA-transposes read fully contiguous DRAM at full xbar bandwidth.
    prep = tc.alloc_tile_pool(name="prep", bufs=4)

    def cast_stage(src, blk, tiles):
        for i in tiles:
            t = prep.tile([128, D], F32, tag="ld_f32")
            nc.sync.dma_start(out=t, in_=src[i * 128:(i + 1) * 128, :])
            tb = prep.tile([128, D], BF16, tag="cast_bf")
            nc.vector.tensor_copy(tb, t)
            for k in range(8):
                nc.scalar.dma_start(
                    out=blk[k, i * 128:(i + 1) * 128, :],
                    in_=tb[:, k * 128:(k + 1) * 128])

    # order: x first (xT unblocks everything), then w_qkv K rows, Q rows,
    # V rows, then w_proj
    cast_stage(x_l, x_blk, range(8))
    cast_stage(wqkv, wqkv_blk, range(8, 16))   # K rows 1024:2048
    cast_stage(wqkv, wqkv_blk, range(0, 8))    # Q rows 0:1024
    cast_stage(wqkv, wqkv_blk, range(16, 24))  # V rows 2048:3072
    cast_stage(wproj, wproj_blk, range(8))

    # bias broadcast-loaded across all 128 partitions (DMA re-reads DRAM row)
    bias_sb = persist1.tile([128, D], F32, tag="bias")
    bias_bcast = bass.AP(tensor=bias.tensor, offset=bias.offset,
                         ap=[[0, 128], *bias.ap])
    nc.sync.dma_start(out=bias_sb, in_=bias_bcast)

    ones_sb = persist1.tile([128, 64], BF16, tag="ones")
    nc.vector.memset(ones_sb, 1.0)

    # persistent attention operands
    qT = [persist1.tile([128, NL], BF16, tag=f"qT{p}", name=f"qT{p}") for p in range(8)]
    kT = [persist1.tile([128, N], BF16, tag=f"kT{p}", name=f"kT{p}") for p in range(8)]
    vv = [persist1.tile([128, D], BF16, tag=f"v{kt}", name=f"v{kt}") for kt in range(16)]

    # ---- Phase B/C: transposed loads + QKV projections --------------------
    qkvp = tc.alloc_tile_pool(name="qkvp", bufs=1)
    qkvsb = tc.alloc_tile_pool(name="qkvsb", bufs=3)

    xT = [qkvp.tile([128, NL], BF16, tag=f"xT{k}", name=f"xT{k}") for k in range(8)]
    for k in range(8):
        for c in range(2):  # 512-token chunks, contiguous DRAM source
            nc.sync.dma_start_transpose(
                out=xT[k][:, c * 512:(c + 1) * 512],
                in_=x_blk[k, c * 512:(c + 1) * 512, :])

    wT = [qkvp.tile([128, 3 * D], BF16, tag=f"wT{k}", name=f"wT{k}") for k in range(8)]

    def wT_load(lo, hi):
        for r0 in range(lo, hi, 512):
            for k in range(8):
                nc.sync.dma_start_transpose(
                    out=wT[k][:, r0:r0 + 512],
                    in_=wqkv_blk[k, r0:r0 + 512, :])

    def proj_dT(m, dst_sb):
        # dst_sb[:, :] = (w_qkv rows m*128..)^T @ x^T  -> [d_out 128, NL]
        for qc in range(2):
            ps = psum.tile([128, 512], F32, tag="qkv_ps", bufs=2)
            for k in range(8):
                nc.tensor.matmul(
                    out=ps,
                    lhsT=wT[k][:, m * 128:(m + 1) * 128],
                    rhs=xT[k][:, qc * 512:(qc + 1) * 512],
                    start=(k == 0), stop=(k == 7))
            nc.vector.tensor_copy(dst_sb[:, qc * 512:(qc + 1) * 512], ps)

    # K projection first so the K AllGather launches as early as possible
    wT_load(1024, 2048)
    for m in range(8, 16):
        ksb = qkvsb.tile([128, NL], BF16, tag="k_loc")
        proj_dT(m, ksb)
        nc.sync.dma_start(out=cc_k[(m - 8) * 128:(m - 7) * 128, :], in_=ksb)
    nc.gpsimd.collective_compute(
        "AllGather", mybir.AluOpType.bypass, replica_groups=RG,
        ins=[cc_k], outs=[k_g])
    # gathered K loads: rank 0 = tokens 0:NL, rank 1 = NL:N (all cores agree)
    for p in range(8):
        nc.sync.dma_start(out=kT[p][:, 0:NL], in_=k_g[0, p * 128:(p + 1) * 128, :])
        nc.sync.dma_start(out=kT[p][:, NL:N], in_=k_g[1, p * 128:(p + 1) * 128, :])

    # Q projection (overlaps the K gather)
    wT_load(0, 1024)
    for m in range(8):
        proj_dT(m, qT[m])

    # V projection, natural [token, d] orientation
    wT_load(2048, 3072)
    for t in range(8):
        vsb = qkvsb.tile([128, D], BF16, tag="v_loc")
        for vc in range(2):
            ps = psum.tile([128, 512], F32, tag="qkv_ps", bufs=2)
            for k in range(8):
                nc.tensor.matmul(
                    out=ps,
                    lhsT=xT[k][:, t * 128:(t + 1) * 128],
                    rhs=wT[k][:, 2 * D + vc * 512:2 * D + (vc + 1) * 512],
                    start=(k == 0), stop=(k == 7))
            nc.vector.tensor_copy(vsb[:, vc * 512:(vc + 1) * 512], ps)
        nc.sync.dma_start(out=cc_v[t * 128:(t + 1) * 128, :], in_=vsb)
    nc.gpsimd.collective_compute(
        "AllGather", mybir.AluOpType.bypass, replica_groups=RG,
        ins=[cc_v], outs=[v_g])
    for kt in range(16):
        nc.sync.dma_start(
            out=vv[kt], in_=v_g[kt // 8, (kt % 8) * 128:(kt % 8 + 1) * 128, :])

    # release staging pools; outstanding QKV instructions still execute, and
    # later pools that reuse these addresses pick up overlap dependencies
    qkvsb.release()
    qkvp.release()
    prep.release()

    # ---- Phase D: attention ----------------------------------------------
    persist2 = tc.alloc_tile_pool(name="persist2", bufs=1)
    attoutT = [persist2.tile([128, NL], BF16, tag=f"ao{p}", name=f"ao{p}") for p in range(8)]
    wpT = [persist2.tile([128, D], BF16, tag=f"wpT{k}", name=f"wpT{k}") for k in range(8)]
    for k in range(8):
        nc.sync.dma_start_transpose(out=wpT[k], in_=wproj_blk[k])

    with tc.tile_pool(name="pT", bufs=4) as ppool, \
         tc.tile_pool(name="rc", bufs=2) as rpool:
        for p in range(8):
            for qc in range(2):
                o = psum.tile([128, 512], F32, tag="o_ps", bufs=1)
                dn = psum.tile([128, 512], F32, tag="den_ps", bufs=1)
                for kt in range(16):
                    s = psum.tile([128, 2, 512], F32, tag="s_ps", bufs=2)
                    for h in range(2):
                        # S^T[k_tok, q] for head 2p+h; contraction over HD=64
                        nc.tensor.matmul(
                            out=s[:, h, :],
                            lhsT=kT[p][h * 64:(h + 1) * 64, kt * 128:(kt + 1) * 128],
                            rhs=qT[p][h * 64:(h + 1) * 64, qc * 512:(qc + 1) * 512],
                            start=True, stop=True,
                            tile_position=(h * 64, 0))
                    pt = ppool.tile([128, 2, 512], BF16, tag="pT")
                    nc.scalar.activation(pt, s, EXP, scale=SCALE)
                    for h in range(2):
                        nc.tensor.matmul(
                            out=o[h * 64:(h + 1) * 64, :],
                            lhsT=vv[kt][:, (2 * p + h) * 64:(2 * p + h + 1) * 64],
                            rhs=pt[:, h, :],
                            start=(kt == 0), stop=(kt == 15),
                            tile_position=(0, h * 64))
                    for h in range(2):
                        nc.tensor.matmul(
                            out=dn[h * 64:(h + 1) * 64, :],
                            lhsT=ones_sb,
                            rhs=pt[:, h, :],
                            start=(kt == 0), stop=(kt == 15),
                            tile_position=(0, h * 64))
                rc = rpool.tile([128, 512], F32, tag="rc")
                nc.vector.reciprocal(rc, dn)
                nc.vector.tensor_mul(attoutT[p][:, qc * 512:(qc + 1) * 512], o, rc)

    # ---- Phase E: output projection + bias --------------------------------
    with tc.tile_pool(name="y_sb", bufs=3) as ypool:
        for tt in range(8):
            for ec in range(2):
                ps = psum.tile([128, 512], F32, tag="qkv_ps", bufs=2)
                for p in range(8):
                    nc.tensor.matmul(
                        out=ps,
                        lhsT=attoutT[p][:, tt * 128:(tt + 1) * 128],
                        rhs=wpT[p][:, ec * 512:(ec + 1) * 512],
                        start=(p == 0), stop=(p == 7))
                yt = ypool.tile([128, 512], F32, tag="y_sb")
                nc.vector.tensor_add(yt, ps, bias_sb[:, ec * 512:(ec + 1) * 512])
                nc.sync.dma_start(
                    out=out[tt * 128:(tt + 1) * 128, ec * 512:(ec + 1) * 512],
                    in_=yt)
    persist2.release()
    psum.release()
    persist1.release()


def _build():
    nc = bacc.Bacc("TRN2", target_bir_lowering=False, debug=False,
                   num_devices=NCORES)
    aps = {
        "x_local": nc.dram_tensor("x_local", [NL, D], F32, kind="ExternalInput").ap(),
        "w_qkv": nc.dram_tensor("w_qkv", [3 * D, D], F32, kind="ExternalInput").ap(),
        "w_proj": nc.dram_tensor("w_proj", [D, D], F32, kind="ExternalInput").ap(),
        "b_proj": nc.dram_tensor("b_proj", [D], F32, kind="ExternalInput").ap(),
        "out": nc.dram_tensor("out", [NL, D], F32, kind="ExternalOutput").ap(),
        "wqkv_blk": nc.dram_tensor("wqkv_blk", [8, 3 * D, 128], BF16).ap(),
        "wproj_blk": nc.dram_tensor("wproj_blk", [8, D, 128], BF16).ap(),
        "x_blk": nc.dram_tensor("x_blk", [8, NL, 128], BF16).ap(),
        "cc_k": nc.dram_tensor("cc_k", [D, NL], BF16).ap(),
        "cc_v": nc.dram_tensor("cc_v", [NL, D], BF16).ap(),
        "k_g": nc.dram_tensor("k_g", [2, D, NL], BF16).ap(),
        "v_g": nc.dram_tensor("v_g", [2, NL, D], BF16).ap(),
    }
    with tile.TileContext(nc) as tc:
        _emit(tc, aps)
    nc.compile()
    return nc


_NC = None


def _get_nc():
    global _NC
    if _NC is None:
        _NC = _build()
    return _NC


def run(x, w_qkv, w_proj, b_proj, **spmd_kwargs):
    nc = _get_nc()
    x = np.ascontiguousarray(np.asarray(x, dtype=np.float32))
    w_qkv = np.ascontiguousarray(np.asarray(w_qkv, dtype=np.float32))
    w_proj = np.ascontiguousarray(np.asarray(w_proj, dtype=np.float32))
    b_proj = np.ascontiguousarray(np.asarray(b_proj, dtype=np.float32))
    in_maps = []
    for c in range(NCORES):
        b, half = divmod(c, 2)
        in_maps.append({
            "x_local": np.ascontiguousarray(x[b, half * NL:(half + 1) * NL, :]),
            "w_qkv": w_qkv,
            "w_proj": w_proj,
            "b_proj": b_proj,
        })
    res = run_bass_kernel_spmd(nc, in_maps, list(range(NCORES)), **spmd_kwargs)
    y = np.empty((B, N, D), dtype=np.float32)
    for c in range(NCORES):
        b, half = divmod(c, 2)
        y[b, half * NL:(half + 1) * NL, :] = res.results[c]["out"]
    return y, res


def kernel(x, w_qkv, w_proj, b_proj):
    y, _ = run(x, w_qkv, w_proj, b_proj)
    return y


# revision 12
# speedup vs baseline: 1.1842x; 1.1842x over previous
"""Multi-head attention (B=4, N=2048, D=1024, H=16) on 8 TRN2 NeuronCores.

Sharding: 8 cores = batch(4) x sequence-half(2). Each core computes the full
attention output for its 1024-token slice of one batch (all 16 heads), so the
final unshard is a pure gather. The only cross-core traffic is an AllGather of
K^T and V between the two cores of each batch pair.

Per-core pipeline (bf16 matmul operands, fp32 PSUM accumulation):
  1. Cast x / w_qkv / w_proj to bf16, stage to DRAM, and DMA-transpose back so
     contraction dims sit on SBUF partitions.
  2. QKV projection. Q^T and K^T are produced in [d_out, token] orientation
     (lhsT = w_qkv^T tile, rhs = x^T); V in natural [token, d] orientation
     (lhsT = x^T tile, rhs = w_qkv^T).
  3. AllGather K^T then V across the pair (k-token axis spans both halves).
  4. Attention per head-pair p: S^T = (QK^T)^T via row-paired matmuls
     (contraction = head_dim 64, two heads in array row halves), exp on
     ScalarE straight out of PSUM (logits are bounded, no max subtraction),
     then O^T and the softmax denominator via col-paired matmuls over the
     k axis. The all-ones denominator lhsT replicates each head's denominator
     across its 64 output partitions, so normalization is a single full-width
     reciprocal + multiply on VectorE.
  5. Output projection from the accumulated attout^T tiles, bias add, DMA out.
"""

import sys

for _p in ("/opt/trn_rl_repo",):
    if _p not in sys.path:
        sys.path.insert(0, _p)

import numpy as np

import concourse.bass as bass
import concourse.mybir as mybir
import concourse.tile as tile
from concourse import bacc
from concourse.bass_utils import run_bass_kernel_spmd

B, N, D, H, HD = 4, 2048, 1024, 16, 64
SCALE = HD ** -0.5
NL = N // 2  # tokens per core
NCORES = 8
RG = [[0, 1], [2, 3], [4, 5], [6, 7]]
F32 = mybir.dt.float32
BF16 = mybir.dt.bfloat16
EXP = mybir.ActivationFunctionType.Exp


def _emit(tc, aps):
    nc = tc.nc
    x_l, wqkv, wproj, bias, out = (
        aps["x_local"], aps["w_qkv"], aps["w_proj"], aps["b_proj"], aps["out"])
    x_blk, wqkv_blk, wproj_blk = aps["x_blk"], aps["wqkv_blk"], aps["wproj_blk"]
    cc_k, cc_v, k_g, v_g = aps["cc_k"], aps["cc_v"], aps["k_g"], aps["v_g"]

    persist1 = tc.alloc_tile_pool(name="persist1", bufs=1)
    psum = tc.alloc_tile_pool(name="psum", bufs=1, space="PSUM")

    # ---- Phase A: load fp32, cast bf16, stage to DRAM in column-blocked
    # layout (one [rows, 128] contiguous block per k-tile) so the later
    # DMA-transposes read fully contiguous DRAM at full xbar bandwidth.
    prep = tc.alloc_tile_pool(name="prep", bufs=4)

    def cast_stage(src, blk, tiles):
        for i in tiles:
            t = prep.tile([128, D], F32, tag="ld_f32")
            nc.sync.dma_start(out=t, in_=src[i * 128:(i + 1) * 128, :])
            tb = prep.tile([128, D], BF16, tag="cast_bf")
            nc.vector.tensor_copy(tb, t)
            # one DMA: [128, 8, 128] sbuf -> 8 column-blocks in DRAM
            dst = bass.AP(tensor=blk.tensor,
                          offset=blk.offset + i * 128 * 128,
                          ap=[[128, 128], [blk.ap[0][0], 8], [1, 128]])
            nc.sync.dma_start(out=dst, in_=tb.rearrange("p (k c) -> p k c", k=8))

    # order: x first (xT unblocks everything), then w_qkv K rows, Q rows,
    # V rows, then w_proj
    cast_stage(x_l, x_blk, range(8))
    cast_stage(wqkv, wqkv_blk, range(8, 16))   # K rows 1024:2048
    cast_stage(wqkv, wqkv_blk, range(0, 8))    # Q rows 0:1024
    cast_stage(wqkv, wqkv_blk, range(16, 24))  # V rows 2048:3072
    cast_stage(wproj, wproj_blk, range(8))

    # bias broadcast-loaded across all 128 partitions (DMA re-reads DRAM row)
    bias_sb = persist1.tile([128, D], F32, tag="bias")
    bias_bcast = bass.AP(tensor=bias.tensor, offset=bias.offset,
                         ap=[[0, 128], *bias.ap])
    nc.sync.dma_start(out=bias_sb, in_=bias_bcast)

    ones_sb = persist1.tile([128, 64], BF16, tag="ones")
    nc.vector.memset(ones_sb, 1.0)

    # persistent attention operands
    qT = [persist1.tile([128, NL], BF16, tag=f"qT{p}", name=f"qT{p}") for p in range(8)]
    kT = [persist1.tile([128, N], BF16, tag=f"kT{p}", name=f"kT{p}") for p in range(8)]
    vv = [persist1.tile([128, D], BF16, tag=f"v{kt}", name=f"v{kt}") for kt in range(16)]

    # ---- Phase B/C: transposed loads + QKV projections --------------------
    qkvp = tc.alloc_tile_pool(name="qkvp", bufs=1)
    qkvsb = tc.alloc_tile_pool(name="qkvsb", bufs=3)

    xT = [qkvp.tile([128, NL], BF16, tag=f"xT{k}", name=f"xT{k}") for k in range(8)]
    for k in range(8):
        nc.sync.dma_start_transpose(out=xT[k], in_=x_blk[k])

    wT = [qkvp.tile([128, 3 * D], BF16, tag=f"wT{k}", name=f"wT{k}") for k in range(8)]

    def wT_load(lo, hi):
        for r0 in range(lo, hi, 1024):
            for k in range(8):
                nc.sync.dma_start_transpose(
                    out=wT[k][:, r0:r0 + 1024],
                    in_=wqkv_blk[k, r0:r0 + 1024, :])

    def proj_dT(m, dst_sb):
        # dst_sb[:, :] = (w_qkv rows m*128..)^T @ x^T  -> [d_out 128, NL]
        # qc-inner: each weight load streams both 512-token chunks into two
        # different PSUM banks (no RAW between consecutive matmuls)
        ps = psum.tile([128, 2, 512], F32, tag="qkv_ps", bufs=1)
        for k in range(8):
            for qc in range(2):
                nc.tensor.matmul(
                    out=ps[:, qc, :],
                    lhsT=wT[k][:, m * 128:(m + 1) * 128],
                    rhs=xT[k][:, qc * 512:(qc + 1) * 512],
                    start=(k == 0), stop=(k == 7))
        for qc in range(2):
            nc.vector.tensor_copy(dst_sb[:, qc * 512:(qc + 1) * 512], ps[:, qc, :])

    # K projection first so the K AllGather launches as early as possible
    wT_load(1024, 2048)
    for m in range(8, 16):
        ksb = qkvsb.tile([128, NL], BF16, tag="k_loc")
        proj_dT(m, ksb)
        nc.sync.dma_start(out=cc_k[(m - 8) * 128:(m - 7) * 128, :], in_=ksb)
    nc.gpsimd.collective_compute(
        "AllGather", mybir.AluOpType.bypass, replica_groups=RG,
        ins=[cc_k], outs=[k_g])
    # gathered K loads: rank 0 = tokens 0:NL, rank 1 = NL:N (all cores agree)
    for p in range(8):
        nc.sync.dma_start(out=kT[p][:, 0:NL], in_=k_g[0, p * 128:(p + 1) * 128, :])
        nc.sync.dma_start(out=kT[p][:, NL:N], in_=k_g[1, p * 128:(p + 1) * 128, :])

    # Q projection (overlaps the K gather)
    wT_load(0, 1024)
    for m in range(8):
        proj_dT(m, qT[m])

    # V projection, natural [token, d] orientation
    wT_load(2048, 3072)
    for t in range(8):
        vsb = qkvsb.tile([128, D], BF16, tag="v_loc")
        ps = psum.tile([128, 2, 512], F32, tag="qkv_ps", bufs=1)
        for k in range(8):
            for vc in range(2):
                nc.tensor.matmul(
                    out=ps[:, vc, :],
                    lhsT=xT[k][:, t * 128:(t + 1) * 128],
                    rhs=wT[k][:, 2 * D + vc * 512:2 * D + (vc + 1) * 512],
                    start=(k == 0), stop=(k == 7))
        for vc in range(2):
            nc.vector.tensor_copy(vsb[:, vc * 512:(vc + 1) * 512], ps[:, vc, :])
        nc.sync.dma_start(out=cc_v[t * 128:(t + 1) * 128, :], in_=vsb)
    nc.gpsimd.collective_compute(
        "AllGather", mybir.AluOpType.bypass, replica_groups=RG,
        ins=[cc_v], outs=[v_g])
    for kt in range(16):
        nc.sync.dma_start(
            out=vv[kt], in_=v_g[kt // 8, (kt % 8) * 128:(kt % 8 + 1) * 128, :])

    # release staging pools; outstanding QKV instructions still execute, and
    # later pools that reuse these addresses pick up overlap dependencies
    qkvsb.release()
    qkvp.release()
    prep.release()

    # ---- Phase D: attention ----------------------------------------------
    persist2 = tc.alloc_tile_pool(name="persist2", bufs=1)
    attoutT = [persist2.tile([128, NL], BF16, tag=f"ao{p}", name=f"ao{p}") for p in range(8)]
    wpT = [persist2.tile([128, D], BF16, tag=f"wpT{k}", name=f"wpT{k}") for k in range(8)]
    for k in range(8):
        nc.sync.dma_start_transpose(out=wpT[k], in_=wproj_blk[k])

    with tc.tile_pool(name="pT", bufs=4) as ppool, \
         tc.tile_pool(name="rc", bufs=2) as rpool:
        for p in range(8):
            for qc in range(2):
                o = psum.tile([128, 512], F32, tag="o_ps", bufs=1)
                dn = psum.tile([128, 512], F32, tag="den_ps", bufs=1)
                for kt in range(16):
                    s = psum.tile([128, 2, 512], F32, tag="s_ps", bufs=2)
                    for h in range(2):
                        # S^T[k_tok, q] for head 2p+h; contraction over HD=64
                        nc.tensor.matmul(
                            out=s[:, h, :],
                            lhsT=kT[p][h * 64:(h + 1) * 64, kt * 128:(kt + 1) * 128],
                            rhs=qT[p][h * 64:(h + 1) * 64, qc * 512:(qc + 1) * 512],
                            start=True, stop=True,
                            tile_position=(h * 64, 0))
                    pt = ppool.tile([128, 2, 512], BF16, tag="pT")
                    nc.scalar.activation(pt, s, EXP, scale=SCALE)
                    for h in range(2):
                        nc.tensor.matmul(
                            out=o[h * 64:(h + 1) * 64, :],
                            lhsT=vv[kt][:, (2 * p + h) * 64:(2 * p + h + 1) * 64],
                            rhs=pt[:, h, :],
                            start=(kt == 0), stop=(kt == 15),
                            tile_position=(0, h * 64))
                    for h in range(2):
                        nc.tensor.matmul(
                            out=dn[h * 64:(h + 1) * 64, :],
                            lhsT=ones_sb,
                            rhs=pt[:, h, :],
                            start=(kt == 0), stop=(kt == 15),
                            tile_position=(0, h * 64))
                rc = rpool.tile([128, 512], F32, tag="rc")
                nc.vector.reciprocal(rc, dn)
                nc.vector.tensor_mul(attoutT[p][:, qc * 512:(qc + 1) * 512], o, rc)

    # ---- Phase E: output projection + bias --------------------------------
    with tc.tile_pool(name="y_sb", bufs=3) as ypool:
        for tt in range(8):
            ps = psum.tile([128, 2, 512], F32, tag="qkv_ps", bufs=1)
            for p in range(8):
                for ec in range(2):
                    nc.tensor.matmul(
                        out=ps[:, ec, :],
                        lhsT=attoutT[p][:, tt * 128:(tt + 1) * 128],
                        rhs=wpT[p][:, ec * 512:(ec + 1) * 512],
                        start=(p == 0), stop=(p == 7))
            yt = ypool.tile([128, D], F32, tag="y_sb")
            for ec in range(2):
                nc.vector.tensor_add(yt[:, ec * 512:(ec + 1) * 512], ps[:, ec, :],
                                     bias_sb[:, ec * 512:(ec + 1) * 512])
            nc.sync.dma_start(out=out[tt * 128:(tt + 1) * 128, :], in_=yt)
    persist2.release()
    psum.release()
    persist1.release()


def _build():
    nc = bacc.Bacc("TRN2", target_bir_lowering=False, debug=False,
                   num_devices=NCORES)
    aps = {
        "x_local": nc.dram_tensor("x_local", [NL, D], F32, kind="ExternalInput").ap(),
        "w_qkv": nc.dram_tensor("w_qkv", [3 * D, D], F32, kind="ExternalInput").ap(),
        "w_proj": nc.dram_tensor("w_proj", [D, D], F32, kind="ExternalInput").ap(),
        "b_proj": nc.dram_tensor("b_proj", [D], F32, kind="ExternalInput").ap(),
        "out": nc.dram_tensor("out", [NL, D], F32, kind="ExternalOutput").ap(),
        "wqkv_blk": nc.dram_tensor("wqkv_blk", [8, 3 * D, 128], BF16).ap(),
        "wproj_blk": nc.dram_tensor("wproj_blk", [8, D, 128], BF16).ap(),
        "x_blk": nc.dram_tensor("x_blk", [8, NL, 128], BF16).ap(),
        "cc_k": nc.dram_tensor("cc_k", [D, NL], BF16).ap(),
        "cc_v": nc.dram_tensor("cc_v", [NL, D], BF16).ap(),
        "k_g": nc.dram_tensor("k_g", [2, D, NL], BF16).ap(),
        "v_g": nc.dram_tensor("v_g", [2, NL, D], BF16).ap(),
    }
    with tile.TileContext(nc) as tc:
        _emit(tc, aps)
    nc.compile()
    return nc


_NC = None


def _get_nc():
    global _NC
    if _NC is None:
        _NC = _build()
    return _NC


def run(x, w_qkv, w_proj, b_proj, **spmd_kwargs):
    nc = _get_nc()
    x = np.ascontiguousarray(np.asarray(x, dtype=np.float32))
    w_qkv = np.ascontiguousarray(np.asarray(w_qkv, dtype=np.float32))
    w_proj = np.ascontiguousarray(np.asarray(w_proj, dtype=np.float32))
    b_proj = np.ascontiguousarray(np.asarray(b_proj, dtype=np.float32))
    in_maps = []
    for c in range(NCORES):
        b, half = divmod(c, 2)
        in_maps.append({
            "x_local": np.ascontiguousarray(x[b, half * NL:(half + 1) * NL, :]),
            "w_qkv": w_qkv,
            "w_proj": w_proj,
            "b_proj": b_proj,
        })
    res = run_bass_kernel_spmd(nc, in_maps, list(range(NCORES)), **spmd_kwargs)
    y = np.empty((B, N, D), dtype=np.float32)
    for c in range(NCORES):
        b, half = divmod(c, 2)
        y[b, half * NL:(half + 1) * NL, :] = res.results[c]["out"]
    return y, res


def kernel(x, w_qkv, w_proj, b_proj):
    y, _ = run(x, w_qkv, w_proj, b_proj)
    return y


# revision 14
# speedup vs baseline: 1.2529x; 1.0581x over previous
"""Multi-head attention (B=4, N=2048, D=1024, H=16) on 8 TRN2 NeuronCores.

Sharding: 8 cores = batch(4) x sequence-half(2). Each core computes the full
attention output for its 1024-token slice of one batch (all 16 heads), so the
final unshard is a pure gather. The only cross-core traffic is an AllGather of
K^T and V between the two cores of each batch pair.

Per-core pipeline (bf16 matmul operands, fp32 PSUM accumulation):
  1. Cast x / w_qkv / w_proj to bf16, stage to DRAM, and DMA-transpose back so
     contraction dims sit on SBUF partitions.
  2. QKV projection. Q^T and K^T are produced in [d_out, token] orientation
     (lhsT = w_qkv^T tile, rhs = x^T); V in natural [token, d] orientation
     (lhsT = x^T tile, rhs = w_qkv^T).
  3. AllGather K^T then V across the pair (k-token axis spans both halves).
  4. Attention per head-pair p: S^T = (QK^T)^T via row-paired matmuls
     (contraction = head_dim 64, two heads in array row halves), exp on
     ScalarE straight out of PSUM (logits are bounded, no max subtraction),
     then O^T and the softmax denominator via col-paired matmuls over the
     k axis. The all-ones denominator lhsT replicates each head's denominator
     across its 64 output partitions, so normalization is a single full-width
     reciprocal + multiply on VectorE.
  5. Output projection from the accumulated attout^T tiles, bias add, DMA out.
"""

import sys

for _p in ("/opt/trn_rl_repo",):
    if _p not in sys.path:
        sys.path.insert(0, _p)

import numpy as np

import concourse.bass as bass
import concourse.mybir as mybir
import concourse.tile as tile
from concourse import bacc
from concourse.bass_utils import run_bass_kernel_spmd

B, N, D, H, HD = 4, 2048, 1024, 16, 64
SCALE = HD ** -0.5
NL = N // 2  # tokens per core
NCORES = 8
RG = [[0, 1], [2, 3], [4, 5], [6, 7]]
F32 = mybir.dt.float32
BF16 = mybir.dt.bfloat16
EXP = mybir.ActivationFunctionType.Exp


def _emit(tc, aps):
    nc = tc.nc
    x_l, wqkv, wproj, bias, out = (
        aps["x_local"], aps["w_qkv"], aps["w_proj"], aps["b_proj"], aps["out"])
    x_blk, wqkv_blk, wproj_blk = aps["x_blk"], aps["wqkv_blk"], aps["wproj_blk"]
    cc_k, cc_v, k_g, v_g = aps["cc_k"], aps["cc_v"], aps["k_g"], aps["v_g"]

    persist1 = tc.alloc_tile_pool(name="persist1", bufs=1)

    # ---- Phase A: load fp32, cast bf16, stage to DRAM in column-blocked
    # layout (one [rows, 128] contiguous block per k-tile) so the later
    # DMA-transposes read fully contiguous DRAM at full xbar bandwidth.
    # Loads on sync, stores + half the transposes on scalar (ScalarE is idle
    # until the first exp) to spread HWDGE issue cost (~0.6-1.4us per DMA).
    prep = tc.alloc_tile_pool(name="prep", bufs=4)

    def cast_tiles(src, blk, tiles):
        for i in tiles:
            t = prep.tile([128, D], F32, tag="ld_f32")
            nc.sync.dma_start(out=t, in_=src[i * 128:(i + 1) * 128, :])
            tb = prep.tile([128, D], BF16, tag="cast_bf")
            nc.vector.tensor_copy(tb, t)
            # one DMA: [128, 8, 128] sbuf -> 8 column-blocks in DRAM
            dst = bass.AP(tensor=blk.tensor,
                          offset=blk.offset + i * 128 * 128,
                          ap=[[128, 128], [blk.ap[0][0], 8], [1, 128]])
            nc.scalar.dma_start(out=dst, in_=tb.rearrange("p (k c) -> p k c", k=8))

    # order: x first (xT unblocks everything), then w_qkv K rows, V rows,
    # Q rows, then w_proj
    cast_tiles(x_l, x_blk, range(8))
    cast_tiles(wqkv, wqkv_blk, range(8, 16))   # K rows 1024:2048
    cast_tiles(wqkv, wqkv_blk, range(16, 24))  # V rows 2048:3072
    cast_tiles(wqkv, wqkv_blk, range(0, 8))    # Q rows 0:1024
    cast_tiles(wproj, wproj_blk, range(8))

    # bias broadcast-loaded across all 128 partitions (DMA re-reads DRAM row)
    bias_sb = persist1.tile([128, D], F32, tag="bias")
    bias_bcast = bass.AP(tensor=bias.tensor, offset=bias.offset,
                         ap=[[0, 128], *bias.ap])
    nc.sync.dma_start(out=bias_sb, in_=bias_bcast)

    ones_sb = persist1.tile([128, 64], BF16, tag="ones")
    nc.vector.memset(ones_sb, 1.0)

    # persistent attention operands
    qT = [persist1.tile([128, NL], BF16, tag=f"qT{p}", name=f"qT{p}") for p in range(8)]
    kT = [persist1.tile([128, N], BF16, tag=f"kT{p}", name=f"kT{p}") for p in range(8)]
    vv = [persist1.tile([128, D], BF16, tag=f"v{kt}", name=f"v{kt}") for kt in range(16)]

    # ---- Phase B/C: transposed loads + QKV projections --------------------
    qkvp = tc.alloc_tile_pool(name="qkvp", bufs=1)
    qkvsb = tc.alloc_tile_pool(name="qkvsb", bufs=3)
    qkvps = tc.alloc_tile_pool(name="qkv_ps", bufs=2, space="PSUM")

    xT = [qkvp.tile([128, NL], BF16, tag=f"xT{k}", name=f"xT{k}") for k in range(8)]
    for k in range(8):
        nc.sync.dma_start_transpose(out=xT[k], in_=x_blk[k])

    wT = [qkvp.tile([128, 3 * D], BF16, tag=f"wT{k}", name=f"wT{k}") for k in range(8)]

    def wT_load(lo, hi):
        for r0 in range(lo, hi, 1024):
            for k in range(8):
                nc.sync.dma_start_transpose(
                    out=wT[k][:, r0:r0 + 1024],
                    in_=wqkv_blk[k, r0:r0 + 1024, :])

    def proj_dT(m, dst_sb):
        # dst_sb[:, :] = (w_qkv rows m*128..)^T @ x^T  -> [d_out 128, NL]
        # qc-inner: each weight streams both 512-token chunks into two
        # different PSUM banks (no RAW between consecutive matmuls)
        ps = qkvps.tile([128, 2, 512], F32, tag="qkv_ps")
        for k in range(8):
            for qc in range(2):
                nc.tensor.matmul(
                    out=ps[:, qc, :],
                    lhsT=wT[k][:, m * 128:(m + 1) * 128],
                    rhs=xT[k][:, qc * 512:(qc + 1) * 512],
                    start=(k == 0), stop=(k == 7))
        for qc in range(2):
            nc.vector.tensor_copy(dst_sb[:, qc * 512:(qc + 1) * 512], ps[:, qc, :])

    # K projection first so the K AllGather launches as early as possible
    wT_load(1024, 2048)
    for m in range(8, 16):
        ksb = qkvsb.tile([128, NL], BF16, tag="k_loc")
        proj_dT(m, ksb)
        nc.sync.dma_start(out=cc_k[(m - 8) * 128:(m - 7) * 128, :], in_=ksb)
    nc.gpsimd.collective_compute(
        "AllGather", mybir.AluOpType.bypass, replica_groups=RG,
        ins=[cc_k], outs=[k_g])
    # gathered K loads: rank 0 = tokens 0:NL, rank 1 = NL:N (all cores agree)
    for p in range(8):
        nc.sync.dma_start(out=kT[p][:, 0:NL], in_=k_g[0, p * 128:(p + 1) * 128, :])
        nc.sync.dma_start(out=kT[p][:, NL:N], in_=k_g[1, p * 128:(p + 1) * 128, :])

    # V projection next (so the V AllGather overlaps the Q projection),
    # natural [token, d] orientation
    wT_load(2048, 3072)
    for t in range(8):
        vsb = qkvsb.tile([128, D], BF16, tag="v_loc")
        ps = qkvps.tile([128, 2, 512], F32, tag="qkv_ps")
        for k in range(8):
            for vc in range(2):
                nc.tensor.matmul(
                    out=ps[:, vc, :],
                    lhsT=xT[k][:, t * 128:(t + 1) * 128],
                    rhs=wT[k][:, 2 * D + vc * 512:2 * D + (vc + 1) * 512],
                    start=(k == 0), stop=(k == 7))
        for vc in range(2):
            nc.vector.tensor_copy(vsb[:, vc * 512:(vc + 1) * 512], ps[:, vc, :])
        nc.sync.dma_start(out=cc_v[t * 128:(t + 1) * 128, :], in_=vsb)
    nc.gpsimd.collective_compute(
        "AllGather", mybir.AluOpType.bypass, replica_groups=RG,
        ins=[cc_v], outs=[v_g])
    for kt in range(16):
        nc.sync.dma_start(
            out=vv[kt], in_=v_g[kt // 8, (kt % 8) * 128:(kt % 8 + 1) * 128, :])

    # Q projection (overlaps the V gather; attention starts right after)
    wT_load(0, 1024)
    for m in range(8):
        proj_dT(m, qT[m])

    # release staging pools; outstanding QKV instructions still execute, and
    # later pools that reuse these addresses pick up overlap dependencies
    qkvps.release()
    qkvsb.release()
    qkvp.release()
    prep.release()

    # ---- Phase D: attention ----------------------------------------------
    persist2 = tc.alloc_tile_pool(name="persist2", bufs=1)
    attoutT = [persist2.tile([128, NL], BF16, tag=f"ao{p}", name=f"ao{p}") for p in range(8)]
    wpT = [persist2.tile([128, D], BF16, tag=f"wpT{k}", name=f"wpT{k}") for k in range(8)]
    for k in range(8):
        nc.sync.dma_start_transpose(out=wpT[k], in_=wproj_blk[k])

    attps = tc.alloc_tile_pool(name="att_ps", bufs=2, space="PSUM")
    with tc.tile_pool(name="pT", bufs=4) as ppool, \
         tc.tile_pool(name="rc", bufs=2) as rpool:
        for p in range(8):
            for qc in range(2):
                o = attps.tile([128, 512], F32, tag="o_ps")
                dn = attps.tile([128, 512], F32, tag="den_ps")
                for kt in range(16):
                    s = attps.tile([128, 2, 512], F32, tag="s_ps")
                    for h in range(2):
                        # S^T[k_tok, q] for head 2p+h; contraction over HD=64
                        nc.tensor.matmul(
                            out=s[:, h, :],
                            lhsT=kT[p][h * 64:(h + 1) * 64, kt * 128:(kt + 1) * 128],
                            rhs=qT[p][h * 64:(h + 1) * 64, qc * 512:(qc + 1) * 512],
                            start=True, stop=True,
                            tile_position=(h * 64, 0))
                    pt = ppool.tile([128, 2, 512], BF16, tag="pT")
                    nc.scalar.activation(pt, s, EXP, scale=SCALE)
                    for h in range(2):
                        nc.tensor.matmul(
                            out=o[h * 64:(h + 1) * 64, :],
                            lhsT=vv[kt][:, (2 * p + h) * 64:(2 * p + h + 1) * 64],
                            rhs=pt[:, h, :],
                            start=(kt == 0), stop=(kt == 15),
                            tile_position=(0, h * 64))
                    for h in range(2):
                        nc.tensor.matmul(
                            out=dn[h * 64:(h + 1) * 64, :],
                            lhsT=ones_sb,
                            rhs=pt[:, h, :],
                            start=(kt == 0), stop=(kt == 15),
                            tile_position=(0, h * 64))
                rc = rpool.tile([128, 512], F32, tag="rc")
                nc.vector.reciprocal(rc, dn)
                nc.vector.tensor_mul(attoutT[p][:, qc * 512:(qc + 1) * 512], o, rc)
    attps.release()

    # ---- Phase E: output projection + bias --------------------------------
    with tc.tile_pool(name="proj_ps", bufs=2, space="PSUM") as projps, \
         tc.tile_pool(name="y_sb", bufs=3) as ypool:
        for tt in range(8):
            ps = projps.tile([128, 2, 512], F32, tag="proj_ps")
            for p in range(8):
                for ec in range(2):
                    nc.tensor.matmul(
                        out=ps[:, ec, :],
                        lhsT=attoutT[p][:, tt * 128:(tt + 1) * 128],
                        rhs=wpT[p][:, ec * 512:(ec + 1) * 512],
                        start=(p == 0), stop=(p == 7))
            yt = ypool.tile([128, D], F32, tag="y_sb")
            for ec in range(2):
                nc.vector.tensor_add(yt[:, ec * 512:(ec + 1) * 512], ps[:, ec, :],
                                     bias_sb[:, ec * 512:(ec + 1) * 512])
            nc.sync.dma_start(out=out[tt * 128:(tt + 1) * 128, :], in_=yt)
    persist2.release()
    persist1.release()


def _build():
    nc = bacc.Bacc("TRN2", target_bir_lowering=False, debug=False,
                   num_devices=NCORES)
    aps = {
        "x_local": nc.dram_tensor("x_local", [NL, D], F32, kind="ExternalInput").ap(),
        "w_qkv": nc.dram_tensor("w_qkv", [3 * D, D], F32, kind="ExternalInput").ap(),
        "w_proj": nc.dram_tensor("w_proj", [D, D], F32, kind="ExternalInput").ap(),
        "b_proj": nc.dram_tensor("b_proj", [D], F32, kind="ExternalInput").ap(),
        "out": nc.dram_tensor("out", [NL, D], F32, kind="ExternalOutput").ap(),
        "wqkv_blk": nc.dram_tensor("wqkv_blk", [8, 3 * D, 128], BF16).ap(),
        "wproj_blk": nc.dram_tensor("wproj_blk", [8, D, 128], BF16).ap(),
        "x_blk": nc.dram_tensor("x_blk", [8, NL, 128], BF16).ap(),
        "cc_k": nc.dram_tensor("cc_k", [D, NL], BF16).ap(),
        "cc_v": nc.dram_tensor("cc_v", [NL, D], BF16).ap(),
        "k_g": nc.dram_tensor("k_g", [2, D, NL], BF16).ap(),
        "v_g": nc.dram_tensor("v_g", [2, NL, D], BF16).ap(),
    }
    with tile.TileContext(nc) as tc:
        _emit(tc, aps)
    nc.compile()
    return nc


_NC = None


def _get_nc():
    global _NC
    if _NC is None:
        _NC = _build()
    return _NC


def run(x, w_qkv, w_proj, b_proj, **spmd_kwargs):
    nc = _get_nc()
    x = np.ascontiguousarray(np.asarray(x, dtype=np.float32))
    w_qkv = np.ascontiguousarray(np.asarray(w_qkv, dtype=np.float32))
    w_proj = np.ascontiguousarray(np.asarray(w_proj, dtype=np.float32))
    b_proj = np.ascontiguousarray(np.asarray(b_proj, dtype=np.float32))
    in_maps = []
    for c in range(NCORES):
        b, half = divmod(c, 2)
        in_maps.append({
            "x_local": np.ascontiguousarray(x[b, half * NL:(half + 1) * NL, :]),
            "w_qkv": w_qkv,
            "w_proj": w_proj,
            "b_proj": b_proj,
        })
    res = run_bass_kernel_spmd(nc, in_maps, list(range(NCORES)), **spmd_kwargs)
    y = np.empty((B, N, D), dtype=np.float32)
    for c in range(NCORES):
        b, half = divmod(c, 2)
        y[b, half * NL:(half + 1) * NL, :] = res.results[c]["out"]
    return y, res


def kernel(x, w_qkv, w_proj, b_proj):
    y, _ = run(x, w_qkv, w_proj, b_proj)
    return y
